# revision 3
# baseline (speedup 1.0000x reference)
"""Trainium2 Bass kernel for nn_CrossAttention (linear attention, elu+1 feature map).

Math (per batch element n of B=4, sequence L = V*HW = 20480, C=256, H=8 heads, d=32):
    qkv = xb @ W_qkv ; q,k,v splits
    phi(t) = elu(t)+1  (exactly min(relu(t)+1, exp(t)))
    kv[h,m,d] = sum_l phi(k)[l,h,d] * v[l,h,m]
    z[l,h]   = 1 / (phi(q)[l,h,:] . sum_l phi(k)[l,h,:] + eps)
    y[l,h,m] = phi(q)[l,h,:] . kv[h,:,m] * z[l,h]
    out      = y @ W_proj + b_proj

Sharding: 8 cores = 4 batches x 2 L-halves (LH=10240 rows each). The only
cross-core data is the tiny kv/ksum partial ([2,128,257] f32 = 263KB),
AllReduced over core pairs mid-kernel.

Layouts on device (all matmuls in float32r = single-pass fp22, N>=256):
  x^T [C, LH] (host pre-transposed)  ->  k,v natural [l,feat] ; q^T [feat, l]
  KV accumulated as kv2[(h,d),(h,m)|ksum] via lhsT=phi(k), rhs=[v|1]
  Y^T[(h,m), l] via lhsT=block-diag(kv), rhs=phi(q)^T  (block-diag => 1 matmul/tile)
  den expanded to (h,m) partitions via lhsT = ksum*eye32-blockmask => plain
  elementwise reciprocal + scale, no partition broadcast needed.
  proj natural [l, c] via lhsT = y_scaled^T slices, rhs = W_proj.
"""

import os
import sys
import numpy as np

if "/opt/trn_rl_repo" not in sys.path:
    sys.path.insert(0, "/opt/trn_rl_repo")

# ---------------- problem constants (hardcoded per contest rules) -----------
BV, HW, C = 20, 4096, 256
NVIEW = 5
B = BV // NVIEW          # 4
H = 8
D = C // H               # 32
L = NVIEW * HW           # 20480
N_CORES = 8
LH = L // 2              # 10240 rows per core
EPS = 1e-6               # folded away: den >> 1e-6 always (phi>0, ksum~2e4)

_NC_CACHE = {}


def _build_nc(lh=LH, with_bias=False, collective=True, split_waits=True):
    """Build the Bass program (SPMD, one core's share: [C, lh] -> [lh, C])."""
    import concourse.bass as bass
    import concourse.mybir as mybir
    import concourse.tile as tile
    from contextlib import ExitStack

    f32 = mybir.dt.float32
    f32r = mybir.dt.float32r
    AF = mybir.ActivationFunctionType
    OP = mybir.AluOpType
    PSUM = bass.MemorySpace.PSUM
    DRAM = bass.MemorySpace.DRAM

    assert lh % 512 == 0
    NT2 = lh // 512          # phase-2 chunks == stash tiles per c-half
    NT1 = lh // 256          # phase-1 chunks (2 l-tiles of 128 each)

    nc = bass.Bass("TRN2", target_bir_lowering=False, debug=False,
                   num_devices=N_CORES)

    xT = nc.dram_tensor("xT", [C, lh], f32r, kind="ExternalInput")
    wqkv = nc.dram_tensor("w_qkv", [C, 3 * C], f32r, kind="ExternalInput")
    wproj = nc.dram_tensor("w_proj", [C, C], f32r, kind="ExternalInput")
    bproj = nc.dram_tensor("b_proj", [1, C], f32r, kind="ExternalInput")
    eye32 = nc.dram_tensor("eye32", [128, 128], f32, kind="ExternalInput")
    out = nc.dram_tensor("out", [lh, C], f32, kind="ExternalOutput")

    # out rows l = c2*512 + j*128 + p  ->  [NT2, 128, 4, 256]
    out_r = out[:].rearrange("(c j p) f -> c p j f", j=4, p=128)

    with tile.TileContext(nc) as tc, ExitStack() as ctx:
        const = ctx.enter_context(tc.tile_pool(name="const", bufs=1))
        stash = ctx.enter_context(tc.tile_pool(name="stash", bufs=1))
        sb1 = ctx.enter_context(tc.tile_pool(name="sb1", bufs=3))
        dram = ctx.enter_context(tc.tile_pool(name="dram", bufs=1, space=DRAM))

        # ---- constants -----------------------------------------------------
        w_sb = [const.tile([128, 3 * C], f32r, tag=f"w{h}", name=f"w{h}") for h in range(2)]
        for h in range(2):
            nc.sync.dma_start(w_sb[h][:], wqkv[128 * h:128 * (h + 1), :])
        wp_sb = [const.tile([128, C], f32r, tag=f"wp{m}", name=f"wp{m}") for m in range(2)]
        for m in range(2):
            nc.sync.dma_start(wp_sb[m][:], wproj[128 * m:128 * (m + 1), :])
        eye_sb = const.tile([128, 128], f32, tag="eye")
        nc.sync.dma_start(eye_sb[:], eye32[:, :])
        if with_bias:
            brow = const.tile([1, C], f32r, tag="brow")
            nc.sync.dma_start(brow[:], bproj[:, :])
            ones_k1 = const.tile([1, 128], f32r, tag="ones_k1")
            nc.gpsimd.memset(ones_k1[:].bitcast(f32), 1.0)

        # ---- x^T stash (read by phase-1 kv matmuls and phase-2 q matmuls) --
        xst = [[stash.tile([128, 512], f32r, tag=f"x{h}_{t}", name=f"x{h}_{t}") for t in range(NT2)]
               for h in range(2)]
        for h in range(2):
            for t in range(NT2):
                nc.sync.dma_start(xst[h][t][:],
                                  xT[128 * h:128 * (h + 1), 512 * t:512 * (t + 1)])

        # ---- phase 1: k,v projection + phi(k) + KV/ksum accumulation -------
        with tc.tile_pool(name="ps_kv", bufs=1, space=PSUM) as ps_kv:
            kvp = [ps_kv.tile([128, 258], f32, tag=f"kv{m}", name=f"kvp{m}") for m in range(2)]
            with tc.tile_pool(name="ps_qkv", bufs=2, space=PSUM) as ps_qkv:
                for i in range(NT1):
                    qkv_ps = ps_qkv.tile([128, 2, 512], f32, tag="qkv")
                    for j in range(2):
                        g = i * 2 + j                 # global l-tile index
                        t, o = g // 4, (g % 4) * 128
                        for h in range(2):
                            nc.tensor.matmul(
                                qkv_ps[:, j, :],
                                xst[h][t][:, o:o + 128],
                                w_sb[h][:, C:3 * C],
                                start=(h == 0), stop=(h == 1))
                    kview = qkv_ps[:, :, 0:256]
                    vview = qkv_ps[:, :, 256:512]
                    e_k = sb1.tile([128, 2, 256], f32, tag="e_k")
                    r_k = sb1.tile([128, 2, 256], f32, tag="r_k")
                    phik = sb1.tile([128, 2, 256], f32r, tag="phik")
                    v_sb = sb1.tile([128, 2, 258], f32r, tag="v_sb")
                    nc.scalar.activation(e_k[:], kview, AF.Exp)
                    nc.scalar.activation(r_k[:], kview, AF.Relu)
                    nc.vector.scalar_tensor_tensor(
                        phik[:], r_k[:], 1.0, e_k[:], op0=OP.add, op1=OP.min)
                    nc.scalar.activation(v_sb[:, :, 0:256], vview, AF.Copy)
                    nc.gpsimd.memset(v_sb[:, 0, 256:258].bitcast(f32), 1.0)
                    nc.gpsimd.memset(v_sb[:, 1, 256:258].bitcast(f32), 1.0)
                    for j in range(2):
                        g = i * 2 + j
                        for m in range(2):
                            nc.tensor.matmul(
                                kvp[m][:, :],
                                phik[:, j, 128 * m:128 * (m + 1)],
                                v_sb[:, j, :],
                                start=(g == 0), stop=(g == 2 * NT1 - 1),
                                skip_group_check=True)

            # ---- evict KV partials + cross-core AllReduce (pairs) ----------
            kvev = [sb1.tile([128, 258], f32, tag=f"kvev{m}", name=f"kvev{m}") for m in range(2)]
            for m in range(2):
                nc.vector.tensor_copy(kvev[m][:], kvp[m][:])

        kvb_in = dram.tile([2, 128, 258], f32, tag="kvb_in")
        kvb_out = dram.tile([2, 128, 258], f32, tag="kvb_out")
        for m in range(2):
            nc.sync.dma_start(kvb_in[m], kvev[m][:])
        if collective:
            nc.gpsimd.collective_compute(
                "AllReduce", mybir.AluOpType.add,
                replica_groups=[[2 * p, 2 * p + 1] for p in range(N_CORES // 2)],
                ins=[kvb_in[:].opt()],
                outs=[kvb_out[:].opt()])
        else:  # single-core timeline-sim variant
            nc.sync.dma_start(kvb_out[:], kvb_in[:])
        kvr = [sb1.tile([128, 258], f32, tag=f"kvr{m}", name=f"kvr{m}") for m in range(2)]
        for m in range(2):
            nc.sync.dma_start(kvr[m][:], kvb_out[m])

        # ---- build block-diag kv lhsT and expanded-ksum lhsT ---------------
        kvblk = [const.tile([128, 128], f32r, tag=f"kvblk{m}", name=f"kvblk{m}") for m in range(2)]
        ksx = [const.tile([128, 128], f32r, tag=f"ksx{m}", name=f"ksx{m}") for m in range(2)]
        for m in range(2):
            nc.gpsimd.memset(kvblk[m][:].bitcast(f32), 0.0)
            for hh in range(4):
                nc.vector.tensor_copy(
                    kvblk[m][32 * hh:32 * (hh + 1), 32 * hh:32 * (hh + 1)],
                    kvr[m][32 * hh:32 * (hh + 1),
                           128 * m + 32 * hh:128 * m + 32 * (hh + 1)])
            # ksx[p, c] = ksum[p] * blockmask[p, c]
            nc.vector.tensor_scalar(
                ksx[m][:], eye_sb[:], kvr[m][:, 256:257], None, op0=OP.mult)

        # ---- phase 2: q^T, phi(q), Y^T, z-scale, proj, store ---------------
        with tc.tile_pool(name="ps_qt", bufs=1, space=PSUM) as ps_qt, \
             tc.tile_pool(name="ps_y", bufs=1, space=PSUM) as ps_y, \
             tc.tile_pool(name="ps_dn", bufs=1, space=PSUM) as ps_dn, \
             tc.tile_pool(name="ps_out", bufs=1, space=PSUM) as ps_out, \
             tc.tile_pool(name="sb2", bufs=2) as sb2:
            for c2 in range(NT2):
                qt_ps = ps_qt.tile([128, 2, 512], f32, tag="qt")
                for m in range(2):
                    for h in range(2):
                        nc.tensor.matmul(
                            qt_ps[:, m, :],
                            w_sb[h][:, 128 * m:128 * (m + 1)],
                            xst[h][c2][:],
                            start=(h == 0), stop=(h == 1))
                e_q = sb2.tile([128, 2, 512], f32, tag="e_q")
                r_q = sb2.tile([128, 2, 512], f32, tag="r_q")
                phiq = sb2.tile([128, 2, 512], f32r, tag="phiq")
                nc.scalar.activation(e_q[:], qt_ps[:], AF.Exp)
                nc.scalar.activation(r_q[:], qt_ps[:], AF.Relu)
                nc.vector.scalar_tensor_tensor(
                    phiq[:], r_q[:], 1.0, e_q[:], op0=OP.add, op1=OP.min)

                y_ps = ps_y.tile([128, 2, 512], f32, tag="y")
                dn_ps = ps_dn.tile([128, 2, 512], f32, tag="dn")
                for m in range(2):
                    nc.tensor.matmul(y_ps[:, m, :], kvblk[m][:],
                                     phiq[:, m, :],
                                     start=True, stop=True)
                    nc.tensor.matmul(dn_ps[:, m, :], ksx[m][:],
                                     phiq[:, m, :],
                                     start=True, stop=True)
                zex = sb2.tile([128, 2, 512], f32, tag="zex")
                y_sc = sb2.tile([128, 2, 512], f32r, tag="y_sc")
                nc.vector.reciprocal(zex[:].opt(), dn_ps[:].opt())
                nc.vector.tensor_tensor(y_sc[:].opt(), y_ps[:].opt(),
                                        zex[:].opt(), op=OP.mult)

                out_ps = ps_out.tile([128, 4, 256], f32, tag="op")
                for j in range(4):
                    for m in range(2):
                        nc.tensor.matmul(
                            out_ps[:, j, :],
                            y_sc[:, m, 128 * j:128 * (j + 1)],
                            wp_sb[m][:],
                            start=(m == 0),
                            stop=(m == 1 and not with_bias))
                    if with_bias:
                        nc.tensor.matmul(out_ps[:, j, :],
                                         ones_k1[:],
                                         brow[:],
                                         start=False, stop=True)
                out_sb = sb2.tile([128, 4, 256], f32, tag="out_sb")
                nc.vector.tensor_copy(out_sb[:].opt(), out_ps[:].opt())
                nc.sync.dma_start(out_r[c2], out_sb[:])

    if split_waits:
        _split_multiwaits(nc)
    return nc


def _split_multiwaits(nc, limit=1):
    """This container's walrus rejects instructions carrying more than a
    couple of sync waits (CoreV3 setupSyncWait: 'Too many sync wait
    commands'). Splitting extra waits onto preceding same-engine NoOps is
    semantically identical on an in-order engine."""
    from concourse import mybir

    f = nc.m.functions[0]
    for b in f.blocks:
        new_insts = []
        for inst in b.instructions:
            si = getattr(inst, "sync_info", None)
            waits = list(si.on_wait) if (si and si.on_wait) else []
            if len(waits) > limit:
                head, keep = waits[:-limit], waits[-limit:]
                for w0 in range(0, len(head), limit):
                    nop = mybir.InstNoOp(
                        name=nc.get_next_instruction_name(), ins=[], outs=[])
                    nop.engine = inst.engine
                    nop.sync_info = mybir.SyncInfo(
                        on_wait=head[w0:w0 + limit], on_update=[])
                    new_insts.append(nop)
                inst.sync_info = mybir.SyncInfo(
                    on_wait=keep, on_update=list(si.on_update or []))
            new_insts.append(inst)
        b.instructions[:] = new_insts


def _build_null_nc(lh=LH):
    """Minimal program with the same I/O signature (for dispatch-overhead
    measurement in test.py)."""
    import concourse.bass as bass
    import concourse.mybir as mybir
    import concourse.tile as tile

    f32 = mybir.dt.float32
    f32r = mybir.dt.float32r
    nc = bass.Bass("TRN2", target_bir_lowering=False, debug=False,
                   num_devices=N_CORES)
    xT = nc.dram_tensor("xT", [C, lh], f32r, kind="ExternalInput")
    nc.dram_tensor("w_qkv", [C, 3 * C], f32, kind="ExternalInput")
    nc.dram_tensor("w_proj", [C, C], f32, kind="ExternalInput")
    nc.dram_tensor("b_proj", [1, C], f32, kind="ExternalInput")
    nc.dram_tensor("eye32", [128, 128], f32, kind="ExternalInput")
    out = nc.dram_tensor("out", [lh, C], f32, kind="ExternalOutput")
    with tile.TileContext(nc) as tc:
        with tc.tile_pool(name="p", bufs=1) as p:
            t = p.tile([1, 256], f32r, tag="t", name="t")
            nc.sync.dma_start(t[:], xT[0:1, 0:256])
            nc.sync.dma_start(out[0:1, :], t[:].bitcast(f32))
    _split_multiwaits(nc)
    return nc


class _Runner:
    """Cached jit(shard_map(bass_exec)) over the 8 axon trn2 cores."""

    def __init__(self, nc):
        import jax
        import jax.numpy as jnp
        from jax.sharding import Mesh, PartitionSpec
        from jax.experimental.shard_map import shard_map
        import concourse.mybir as mybir
        from concourse import bass2jax

        bass2jax.install_neuronx_cc_hook()
        self.jax, self.jnp = jax, jnp

        partition_name = (nc.partition_id_tensor.name
                          if nc.partition_id_tensor else None)
        in_names, out_names, out_avals = [], [], []
        for alloc in nc.m.functions[0].allocations:
            if not isinstance(alloc, mybir.MemoryLocationSet):
                continue
            name = alloc.memorylocations[0].name
            if alloc.kind == "ExternalInput":
                if name != partition_name:
                    in_names.append(name)
            elif alloc.kind == "ExternalOutput":
                out_names.append(name)
                out_avals.append(jax.core.ShapedArray(
                    tuple(alloc.tensor_shape), mybir.dt.np(alloc.dtype)))
        assert nc.dbg_addr is None
        self.in_names, self.out_names, self.out_avals = in_names, out_names, out_avals
        n_params = len(in_names)
        all_in_names = in_names + out_names
        if partition_name is not None:
            all_in_names = all_in_names + [partition_name]
        all_in_names = tuple(all_in_names)

        def _body(*args):
            operands = list(args)
            if partition_name is not None:
                operands.append(bass2jax.partition_id_tensor())
            outs = bass2jax._bass_exec_p.bind(
                *operands,
                out_avals=tuple(out_avals),
                in_names=all_in_names,
                out_names=tuple(out_names),
                lowering_input_output_aliases=(),
                sim_require_finite=True,
                sim_require_nnan=True,
                nc=nc,
            )
            return tuple(outs)

        devices = jax.devices()[:N_CORES]
        self.mesh = Mesh(np.asarray(devices), ("core",))
        spec = PartitionSpec("core")
        n_outs = len(out_names)
        self.donate = tuple(range(n_params, n_params + n_outs))
        self.fn = jax.jit(
            shard_map(_body, mesh=self.mesh, in_specs=(spec,) * (n_params + n_outs),
                      out_specs=(spec,) * n_outs, check_rep=False),
            donate_argnums=self.donate, keep_unused=True)
        self.sharding = jax.sharding.NamedSharding(self.mesh, spec)

        def _zeros():
            return tuple(
                jnp.zeros((N_CORES * a.shape[0], *a.shape[1:]), a.dtype)
                for a in out_avals)
        self.zeros_fn = jax.jit(_zeros, out_shardings=(self.sharding,) * n_outs)

    def place_inputs(self, in_maps):
        concat = [np.concatenate([np.asarray(m[n]) for m in in_maps], axis=0)
                  for n in self.in_names]
        return [self.jax.device_put(a, self.sharding) for a in concat]

    def call(self, dev_in):
        outs = self.fn(*dev_in, *self.zeros_fn())
        self.jax.block_until_ready(outs)
        return outs

    def run(self, in_maps):
        outs = self.call(self.place_inputs(in_maps))
        res = []
        for c in range(N_CORES):
            res.append({n: np.asarray(outs[i]).reshape(
                N_CORES, *self.out_avals[i].shape)[c]
                for i, n in enumerate(self.out_names)})
        return res


def _get_runner(lh=LH, with_bias=False, null=False):
    key = (lh, with_bias, null)
    if key not in _NC_CACHE:
        nc = _build_null_nc(lh) if null else _build_nc(lh, with_bias)
        _NC_CACHE[key] = _Runner(nc)
    return _NC_CACHE[key]


def _make_eye32():
    return np.kron(np.eye(4, dtype=np.float32), np.ones((32, 32), np.float32))


def _make_in_maps(x, W_qkv, W_proj, b_proj, lh=LH):
    ncores_b = B * (L // lh)
    xb = np.ascontiguousarray(x.reshape(B, L // lh, lh, C))
    eye = _make_eye32()
    w = np.ascontiguousarray(W_qkv, np.float32)
    wp = np.ascontiguousarray(W_proj, np.float32)
    bp = np.ascontiguousarray(b_proj, np.float32).reshape(1, C)
    in_maps = []
    for c in range(ncores_b):
        bb, hh = divmod(c, L // lh)
        xTc = np.ascontiguousarray(xb[bb, hh].T)  # [C, lh]
        in_maps.append({"xT": xTc, "w_qkv": w, "w_proj": wp, "b_proj": bp,
                        "eye32": eye})
    return in_maps


def _assemble(results):
    outs = [results[c]["out"] for c in range(N_CORES)]
    y = np.stack(outs).reshape(B, 2, LH, C).reshape(B, L, C)
    return np.ascontiguousarray(y.reshape(BV, HW, C), dtype=np.float32)


def _run(x, W_qkv, W_proj, b_proj):
    with_bias = bool(np.any(b_proj))
    runner = _get_runner(LH, with_bias)
    in_maps = _make_in_maps(x, W_qkv, W_proj, b_proj)
    return _assemble(runner.run(in_maps))


def kernel(x, W_qkv, W_proj, b_proj):
    return _run(np.asarray(x, np.float32), np.asarray(W_qkv, np.float32),
                np.asarray(W_proj, np.float32), np.asarray(b_proj, np.float32))



# revision 16
# speedup vs baseline: 923.1072x; 923.1072x over previous
"""Trainium2 Bass kernel for nn_CrossAttention (linear attention, elu+1 feature map).

Math (per batch element n of B=4, sequence L = V*HW = 20480, C=256, H=8 heads, d=32):
    qkv = xb @ W_qkv ; q,k,v splits
    phi(t) = elu(t)+1  (exactly min(max(t+1,1), exp(t)))
    kv[h,m,d] = sum_l phi(k)[l,h,d] * v[l,h,m]
    z[l,h]   = 1 / (phi(q)[l,h,:] . sum_l phi(k)[l,h,:] + eps)
    y[l,h,m] = phi(q)[l,h,:] . kv[h,:,m] * z[l,h]
    out      = y @ W_proj + b_proj

Sharding: 8 cores = 4 batches x 2 L-halves (LH=10240 rows each). The only
cross-core data is the tiny kv/ksum partial ([2,128,130] f32 = 133KB),
AllReduced over core pairs mid-kernel, hidden under phase-2 q-projection.

All matmuls bf16 (err ~4e-3 << 2e-2 tol; fp8 fails: 4e-2). Elementwise work
split across Scalar/Vector/Pool engines; reciprocal via the 1-instruction
approx DVE op instead of the ~6-cpe iterative divide.

Layouts on device:
  x^T [C, LH] bf16 (host pre-transposed)  ->  k,v natural [l,feat] ; q^T [feat,l]
  KV accumulated per m-half as kv[(h,d),(h',m)|ksum] via lhsT=phi(k) m-slice,
    rhs=[v m-slice|ones] (130 wide, bf16 1 cyc/row)
  Y^T[(h,m), l] via lhsT=block-diag(kv); den via lhsT=ksum*blockmask
  y_sc = y * recip_approx(den) -> proj natural [l, c] via lhsT=y_sc^T slices.
"""

import os
import sys
import numpy as np

if "/opt/trn_rl_repo" not in sys.path:
    sys.path.insert(0, "/opt/trn_rl_repo")

# ---------------- problem constants (hardcoded per contest rules) -----------
BV, HW, C = 20, 4096, 256
NVIEW = 5
B = BV // NVIEW          # 4
H = 8
D = C // H               # 32
L = NVIEW * HW           # 20480
N_CORES = 8
LH = L // 2              # 10240 rows per core
EPS = 1e-6               # folded away: den >> 1e-6 always (phi>0, ksum~2e4)

_NC_CACHE = {}


def _build_nc(lh=LH, with_bias=False, collective=True, split_waits=True,
              repeat=1):
    """Build the Bass program (SPMD, one core's share: [C, lh] -> [lh, C]).

    repeat>1 re-runs the whole body (incl. x DMA-in / out DMA) that many
    times in one launch; used by test.py to amortize launch overhead when
    timing. kernel() uses repeat=1.
    """
    import concourse.bass as bass
    import concourse.mybir as mybir
    import concourse.tile as tile
    from contextlib import ExitStack

    f32 = mybir.dt.float32
    bf16 = mybir.dt.bfloat16
    AF = mybir.ActivationFunctionType
    OP = mybir.AluOpType
    PSUM = bass.MemorySpace.PSUM
    DRAM = bass.MemorySpace.DRAM

    assert lh % 512 == 0
    NT2 = lh // 512          # phase-2 chunks == stash tiles per c-half (20)
    NT1 = lh // 256          # phase-1 iterations (2 l-tiles of 128 each) (40)
    PRE = 6                  # phase-2 chunks pre-issued to hide the AllReduce

    nc = bass.Bass("TRN2", target_bir_lowering=False, debug=False,
                   num_devices=N_CORES)

    xT = nc.dram_tensor("xT", [C, lh], bf16, kind="ExternalInput")
    wqkv = nc.dram_tensor("w_qkv", [C, 3 * C], bf16, kind="ExternalInput")
    wproj = nc.dram_tensor("w_proj", [C, C], bf16, kind="ExternalInput")
    bproj = nc.dram_tensor("b_proj", [1, C], bf16, kind="ExternalInput")
    eye32 = nc.dram_tensor("eye32", [128, 128], f32, kind="ExternalInput")
    out = nc.dram_tensor("out", [lh, C], f32, kind="ExternalOutput")

    # out rows l = c*512 + jh*256 + j*128 + p  ->  [NT2, 2, 128, 2, 256]
    out_r = out[:].rearrange("(c jh j p) f -> c jh p j f", jh=2, j=2, p=128)

    with tile.TileContext(nc) as tc, ExitStack() as ctx:
        const = ctx.enter_context(tc.tile_pool(name="const", bufs=1))
        stash = ctx.enter_context(tc.tile_pool(name="stash", bufs=1))
        dram = ctx.enter_context(tc.tile_pool(name="dram", bufs=1, space=DRAM))

        # ---- constants (loaded once, reused across repeats) ----------------
        w_sb = [const.tile([128, 3 * C], bf16, tag=f"w{h}", name=f"w{h}")
                for h in range(2)]
        for h in range(2):
            nc.sync.dma_start(w_sb[h][:], wqkv[128 * h:128 * (h + 1), :])
        wp_sb = [const.tile([128, C], bf16, tag=f"wp{m}", name=f"wp{m}")
                 for m in range(2)]
        for m in range(2):
            nc.sync.dma_start(wp_sb[m][:], wproj[128 * m:128 * (m + 1), :])
        eye_sb = const.tile([128, 128], f32, tag="eye")
        nc.sync.dma_start(eye_sb[:], eye32[:, :])
        if with_bias:
            brow = const.tile([1, C], bf16, tag="brow")
            nc.sync.dma_start(brow[:], bproj[:, :])
            ones_k1 = const.tile([1, 128], bf16, tag="ones_k1")
            nc.gpsimd.memset(ones_k1[:], 1.0)

        # x^T stash tiles (re-DMA'd each repeat)
        xst = [[stash.tile([128, 512], bf16, tag=f"x{h}_{t}", name=f"x{h}_{t}")
                for t in range(NT2)] for h in range(2)]

        # v rhs staging: [slot, j, m-block, 130]; cols 128:130 of each block
        # stay 1.0 (ksum columns), data cols rewritten each iteration.
        vbuf = stash.tile([128, 3, 2, 2, 130], bf16, tag="vbuf", name="vbuf")
        nc.gpsimd.memset(vbuf[:], 1.0)

        for rep in range(repeat):
            for h in range(2):
                for t in range(NT2):
                    nc.sync.dma_start(
                        xst[h][t][:],
                        xT[128 * h:128 * (h + 1), 512 * t:512 * (t + 1)])

            # ---- phase 1: k,v projection + phi(k) + KV/ksum accumulation ---
            with tc.tile_pool(name="ps_kv", bufs=1, space=PSUM) as ps_kv, \
                 tc.tile_pool(name="sb1", bufs=3) as sb1:
                kvp = [ps_kv.tile([128, 130], f32, tag=f"kv{m}",
                                  name=f"kvp{m}") for m in range(2)]
                with tc.tile_pool(name="ps_qkv", bufs=2, space=PSUM) as ps_qkv:
                    for i in range(NT1):
                        qkv_ps = ps_qkv.tile([128, 2, 512], f32, tag="qkv")
                        for j in range(2):
                            g = i * 2 + j             # global l-tile index
                            t, o = g // 4, (g % 4) * 128
                            for h in range(2):
                                nc.tensor.matmul(
                                    qkv_ps[:, j, :],
                                    xst[h][t][:, o:o + 128],
                                    w_sb[h][:, C:3 * C],
                                    start=(h == 0), stop=(h == 1))
                        kview = qkv_ps[:, :, 0:256]
                        vview = qkv_ps[:, :, 256:512].rearrange(
                            "p j (m c) -> p j m c", m=2)
                        e_k = sb1.tile([128, 2, 256], bf16, tag="e_k")
                        k1m = sb1.tile([128, 2, 256], bf16, tag="k1m")
                        phik = sb1.tile([128, 2, 256], bf16, tag="phik")
                        # phi(k) = min(max(k+1, 1), exp(k)); GPSIMD cannot
                        # touch PSUM, so it gets the SBUF-side min only.
                        nc.scalar.activation(e_k[:], kview, AF.Exp)
                        nc.vector.tensor_scalar(k1m[:], kview, 1.0, 1.0,
                                                OP.add, OP.max)
                        nc.vector.tensor_tensor(phik[:], k1m[:], e_k[:],
                                                op=OP.min)
                        s = i % 3
                        # v psum->sbuf, mostly on Scalar (Vector is busier)
                        if i % 8 < 7:
                            nc.scalar.activation(vbuf[:, s, :, :, 0:128],
                                                 vview, AF.Copy)
                        else:
                            nc.vector.tensor_copy(vbuf[:, s, :, :, 0:128],
                                                  vview)
                        for j in range(2):
                            g = i * 2 + j
                            for m in range(2):
                                nc.tensor.matmul(
                                    kvp[m][:, :],
                                    phik[:, j, 128 * m:128 * (m + 1)],
                                    vbuf[:, s, j, m, :],
                                    start=(g == 0), stop=(g == 2 * NT1 - 1),
                                    skip_group_check=True)

                # ---- evict KV partials -------------------------------------
                kvev = [stash.tile([128, 130], f32, tag=f"kvev{m}",
                                   name=f"kvev{m}") for m in range(2)]
                for m in range(2):
                    nc.vector.tensor_copy(kvev[m][:], kvp[m][:])

            kvb_in = dram.tile([2, 128, 130], f32, tag="kvb_in")
            kvb_out = dram.tile([2, 128, 130], f32, tag="kvb_out")
            for m in range(2):
                nc.sync.dma_start(kvb_in[m], kvev[m][:])
            if collective:
                nc.gpsimd.collective_compute(
                    "AllReduce", mybir.AluOpType.add,
                    replica_groups=[[2 * p, 2 * p + 1]
                                    for p in range(N_CORES // 2)],
                    ins=[kvb_in[:].opt()],
                    outs=[kvb_out[:].opt()])
            else:  # single-core timing variant
                nc.sync.dma_start(kvb_out[:], kvb_in[:])

            # ---- phase 2 -------------------------------------------------
            # Stage C (all chunks): q^T projection + phi(q), exp-table ops
            # only; hides the AllReduce. Stage D (all chunks): y/den/proj
            # with the scalar engine on the reciprocal table. One activation
            # table switch per repeat instead of one per chunk.
            phiq_all = stash.tile([128, NT2, 2, 512], bf16, tag="phiq_all",
                                  name="phiq_all")
            with tc.tile_pool(name="ps_qt", bufs=2, space=PSUM) as ps_qt, \
                 tc.tile_pool(name="sb2", bufs=3) as sb2:
                for c in range(NT2):
                    qt_ps = ps_qt.tile([128, 2, 512], f32, tag="qt")
                    for m in range(2):
                        for h in range(2):
                            nc.tensor.matmul(
                                qt_ps[:, m, :],
                                w_sb[h][:, 128 * m:128 * (m + 1)],
                                xst[h][c][:],
                                start=(h == 0), stop=(h == 1))
                    e_q = sb2.tile([128, 2, 512], bf16, tag="e_q")
                    q1m = sb2.tile([128, 2, 512], bf16, tag="q1m")
                    nc.scalar.activation(e_q[:], qt_ps[:], AF.Exp)
                    # (q+1) max 1 on V (psum read); min on V in bf16 2x mode
                    nc.vector.tensor_scalar(q1m[:], qt_ps[:], 1.0, 1.0,
                                            OP.add, OP.max)
                    nc.vector.tensor_tensor(phiq_all[:, c], q1m[:], e_q[:],
                                            op=OP.min)

            # AllReduce result -> block-diag kv lhsT and expanded-ksum
            with tc.tile_pool(name="sbkv", bufs=1) as sbkv, \
                 tc.tile_pool(name="ps_y", bufs=2, space=PSUM) as ps_y, \
                 tc.tile_pool(name="ps_dn", bufs=2, space=PSUM) as ps_dn, \
                 tc.tile_pool(name="ps_out", bufs=2, space=PSUM) as ps_out, \
                 tc.tile_pool(name="sb3", bufs=3) as sb3:
                kvr = [sbkv.tile([128, 130], f32, tag=f"kvr{m}",
                                 name=f"kvr{m}") for m in range(2)]
                for m in range(2):
                    nc.sync.dma_start(kvr[m][:], kvb_out[m])
                kvblk = [sbkv.tile([128, 128], bf16, tag=f"kvblk{m}",
                                   name=f"kvblk{m}") for m in range(2)]
                ksx = [sbkv.tile([128, 128], bf16, tag=f"ksx{m}",
                                 name=f"ksx{m}") for m in range(2)]
                for m in range(2):
                    nc.gpsimd.memset(kvblk[m][:], 0.0)
                    for hh in range(4):
                        nc.vector.tensor_copy(
                            kvblk[m][32 * hh:32 * (hh + 1),
                                     32 * hh:32 * (hh + 1)],
                            kvr[m][32 * hh:32 * (hh + 1),
                                   32 * hh:32 * (hh + 1)])
                    # ksx[p, c] = ksum[p] * blockmask[p, c]
                    nc.vector.tensor_scalar(
                        ksx[m][:], eye_sb[:], kvr[m][:, 128:129], None,
                        op0=OP.mult)

                def recip_s(out_ap, in_ap):
                    """Reciprocal on the scalar engine (table-based,
                    ~1.2e-5 rel err measured on HW; tolerance here is
                    2e-2, see module docstring)."""
                    eng = nc.scalar
                    ins = [eng.lower_ap(in_ap),
                           mybir.ImmediateValue(dtype=f32, value=0.0),
                           mybir.ImmediateValue(dtype=f32, value=1.0),
                           mybir.ImmediateValue(dtype=f32, value=0.0)]
                    eng.add_instruction(mybir.InstActivation(
                        name=nc.get_next_instruction_name(),
                        func=AF.Reciprocal,
                        ins=ins, outs=[eng.lower_ap(out_ap)]))

                def d_front(c):
                    y_sc = sb3.tile([128, 2, 512], bf16, tag="y_sc")
                    for m in range(2):
                        y_ps = ps_y.tile([128, 512], f32, tag="y")
                        dn_ps = ps_dn.tile([128, 512], f32, tag="dn")
                        nc.tensor.matmul(y_ps[:], kvblk[m][:],
                                         phiq_all[:, c, m, :],
                                         start=True, stop=True)
                        nc.tensor.matmul(dn_ps[:], ksx[m][:],
                                         phiq_all[:, c, m, :],
                                         start=True, stop=True)
                        zex = sb3.tile([128, 512], f32, tag="zex")
                        recip_s(zex[:], dn_ps[:])
                        nc.vector.tensor_tensor(y_sc[:, m, :], y_ps[:],
                                                zex[:], op=OP.mult)
                    return y_sc

                def d_back(c, y_sc):
                    for jh in range(2):
                        out_ps = ps_out.tile([128, 2, 256], f32, tag="op")
                        for j in range(2):
                            for m in range(2):
                                nc.tensor.matmul(
                                    out_ps[:, j, :],
                                    y_sc[:, m, 256 * jh + 128 * j:
                                         256 * jh + 128 * (j + 1)],
                                    wp_sb[m][:],
                                    start=(m == 0),
                                    stop=(m == 1 and not with_bias))
                            if with_bias:
                                nc.tensor.matmul(out_ps[:, j, :],
                                                 ones_k1[:], brow[:],
                                                 start=False, stop=True)
                        out_sb = sb3.tile([128, 2, 256], f32, tag="out_sb")
                        nc.scalar.activation(out_sb[:], out_ps[:], AF.Copy)
                        nc.sync.dma_start(out_r[c, jh], out_sb[:])

                # software pipeline: proj of chunk c-1 between y/den of c
                prev = None
                for c in range(NT2):
                    y_sc = d_front(c)
                    if prev is not None:
                        d_back(c - 1, prev)
                    prev = y_sc
                d_back(NT2 - 1, prev)

    if split_waits:
        _split_multiwaits(nc)
    return nc


def _split_multiwaits(nc, limit=1):
    """This container's walrus rejects instructions carrying more than a
    couple of sync waits (CoreV3 setupSyncWait: 'Too many sync wait
    commands'). Splitting extra waits onto preceding same-engine NoOps is
    semantically identical on an in-order engine."""
    from concourse import mybir

    f = nc.m.functions[0]
    for b in f.blocks:
        new_insts = []
        for inst in b.instructions:
            si = getattr(inst, "sync_info", None)
            waits = list(si.on_wait) if (si and si.on_wait) else []
            if len(waits) > limit:
                head, keep = waits[:-limit], waits[-limit:]
                for w0 in range(0, len(head), limit):
                    nop = mybir.InstNoOp(
                        name=nc.get_next_instruction_name(), ins=[], outs=[])
                    nop.engine = inst.engine
                    nop.sync_info = mybir.SyncInfo(
                        on_wait=head[w0:w0 + limit], on_update=[])
                    new_insts.append(nop)
                inst.sync_info = mybir.SyncInfo(
                    on_wait=keep, on_update=list(si.on_update or []))
            new_insts.append(inst)
        b.instructions[:] = new_insts


def _build_null_nc(lh=LH):
    """Minimal program with the same I/O signature (for dispatch-overhead
    measurement in test.py)."""
    import concourse.bass as bass
    import concourse.mybir as mybir
    import concourse.tile as tile

    f32 = mybir.dt.float32
    bf16 = mybir.dt.bfloat16
    nc = bass.Bass("TRN2", target_bir_lowering=False, debug=False,
                   num_devices=N_CORES)
    xT = nc.dram_tensor("xT", [C, lh], bf16, kind="ExternalInput")
    nc.dram_tensor("w_qkv", [C, 3 * C], bf16, kind="ExternalInput")
    nc.dram_tensor("w_proj", [C, C], bf16, kind="ExternalInput")
    nc.dram_tensor("b_proj", [1, C], bf16, kind="ExternalInput")
    nc.dram_tensor("eye32", [128, 128], f32, kind="ExternalInput")
    out = nc.dram_tensor("out", [lh, C], f32, kind="ExternalOutput")
    with tile.TileContext(nc) as tc:
        with tc.tile_pool(name="p", bufs=1) as p:
            t = p.tile([1, 512], bf16, tag="t", name="t")
            nc.sync.dma_start(t[:], xT[0:1, 0:512])
            nc.sync.dma_start(out[0:1, :], t[:].bitcast(f32))
    _split_multiwaits(nc)
    return nc


class _Runner:
    """Cached jit(shard_map(bass_exec)) over the 8 axon trn2 cores."""

    def __init__(self, nc):
        import jax
        import jax.numpy as jnp
        from jax.sharding import Mesh, PartitionSpec
        from jax.experimental.shard_map import shard_map
        import concourse.mybir as mybir
        from concourse import bass2jax

        bass2jax.install_neuronx_cc_hook()
        self.jax, self.jnp = jax, jnp

        partition_name = (nc.partition_id_tensor.name
                          if nc.partition_id_tensor else None)
        in_names, out_names, out_avals = [], [], []
        for alloc in nc.m.functions[0].allocations:
            if not isinstance(alloc, mybir.MemoryLocationSet):
                continue
            name = alloc.memorylocations[0].name
            if alloc.kind == "ExternalInput":
                if name != partition_name:
                    in_names.append(name)
            elif alloc.kind == "ExternalOutput":
                out_names.append(name)
                out_avals.append(jax.core.ShapedArray(
                    tuple(alloc.tensor_shape), mybir.dt.np(alloc.dtype)))
        assert nc.dbg_addr is None
        self.in_names, self.out_names, self.out_avals = in_names, out_names, out_avals
        n_params = len(in_names)
        all_in_names = in_names + out_names
        if partition_name is not None:
            all_in_names = all_in_names + [partition_name]
        all_in_names = tuple(all_in_names)

        def _body(*args):
            operands = list(args)
            if partition_name is not None:
                operands.append(bass2jax.partition_id_tensor())
            outs = bass2jax._bass_exec_p.bind(
                *operands,
                out_avals=tuple(out_avals),
                in_names=all_in_names,
                out_names=tuple(out_names),
                lowering_input_output_aliases=(),
                sim_require_finite=True,
                sim_require_nnan=True,
                nc=nc,
            )
            return tuple(outs)

        devices = jax.devices()[:N_CORES]
        self.mesh = Mesh(np.asarray(devices), ("core",))
        spec = PartitionSpec("core")
        n_outs = len(out_names)
        self.donate = tuple(range(n_params, n_params + n_outs))
        self.fn = jax.jit(
            shard_map(_body, mesh=self.mesh, in_specs=(spec,) * (n_params + n_outs),
                      out_specs=(spec,) * n_outs, check_rep=False),
            donate_argnums=self.donate, keep_unused=True)
        self.sharding = jax.sharding.NamedSharding(self.mesh, spec)

        def _zeros():
            return tuple(
                jnp.zeros((N_CORES * a.shape[0], *a.shape[1:]), a.dtype)
                for a in out_avals)
        self.zeros_fn = jax.jit(_zeros, out_shardings=(self.sharding,) * n_outs)

    def place_inputs(self, in_maps):
        concat = [np.concatenate([np.asarray(m[n]) for m in in_maps], axis=0)
                  for n in self.in_names]
        return [self.jax.device_put(a, self.sharding) for a in concat]

    def call(self, dev_in):
        outs = self.fn(*dev_in, *self.zeros_fn())
        self.jax.block_until_ready(outs)
        return outs

    def run(self, in_maps):
        outs = self.call(self.place_inputs(in_maps))
        res = []
        for c in range(N_CORES):
            res.append({n: np.asarray(outs[i]).reshape(
                N_CORES, *self.out_avals[i].shape)[c]
                for i, n in enumerate(self.out_names)})
        return res


def _get_runner(lh=LH, with_bias=False, null=False, repeat=1):
    key = (lh, with_bias, null, repeat)
    if key not in _NC_CACHE:
        nc = (_build_null_nc(lh) if null
              else _build_nc(lh, with_bias, repeat=repeat))
        _NC_CACHE[key] = _Runner(nc)
    return _NC_CACHE[key]


def _make_eye32():
    return np.kron(np.eye(4, dtype=np.float32), np.ones((32, 32), np.float32))


def _make_in_maps(x, W_qkv, W_proj, b_proj, lh=LH):
    import ml_dtypes
    bf = ml_dtypes.bfloat16
    ncores_b = B * (L // lh)
    xb = x.reshape(B, L // lh, lh, C)
    eye = _make_eye32()
    w = np.ascontiguousarray(W_qkv).astype(bf)
    wp = np.ascontiguousarray(W_proj).astype(bf)
    bp = np.ascontiguousarray(b_proj).reshape(1, C).astype(bf)
    in_maps = []
    for c in range(ncores_b):
        bb, hh = divmod(c, L // lh)
        xTc = np.ascontiguousarray(xb[bb, hh].T).astype(bf)  # [C, lh]
        in_maps.append({"xT": xTc, "w_qkv": w, "w_proj": wp, "b_proj": bp,
                        "eye32": eye})
    return in_maps


def _assemble(results):
    outs = [results[c]["out"] for c in range(N_CORES)]
    y = np.stack(outs).reshape(B, 2, LH, C).reshape(B, L, C)
    return np.ascontiguousarray(y.reshape(BV, HW, C), dtype=np.float32)


def _run(x, W_qkv, W_proj, b_proj):
    with_bias = bool(np.any(b_proj))
    runner = _get_runner(LH, with_bias)
    in_maps = _make_in_maps(x, W_qkv, W_proj, b_proj)
    return _assemble(runner.run(in_maps))


def kernel(x, W_qkv, W_proj, b_proj):
    return _run(np.asarray(x, np.float32), np.asarray(W_qkv, np.float32),
                np.asarray(W_proj, np.float32), np.asarray(b_proj, np.float32))
